# revision 38
# baseline (speedup 1.0000x reference)
"""Involution2d Bass kernel for 8 trn2 NeuronCores.

Sharding: core = 2*b + half  (b = batch 0..3, half = group-half 0..1).
Each core computes out[b, half*128:(half+1)*128, :, :].

Math: ker = A @ x[b] + b_span with A = w_span @ w_reduce folded on host.
out[c,p] = sum_kk ker[g(c),kk,p] * xpad[c, p+delta_kk]

Dataflow per core:
 - ker-gen: bf16 PE matmuls (K=256 in 2 chunks) -> PSUM -> ACT copy+bias
   into ker_sb bf16. Row layout per m-tile: r = g*16 + tt (g-major), where
   tap kk = mt*16 + tt.
 - kerb delivery (broadcast each g-row to its 16 channels):
   * DMA path: per (quarter, m-tile), 16 strided SBUF->SBUF DMAs
     (dst[c16::16] <- ker_sb rows) fill kerbS [128, 16, QPIX] bf16.
   * PE path: selection matmul (K=128) -> PSUM -> ACT copy to bf16.
 - DVE tensor_tensor (bf16 2x mode) multiplies shifted xpad view by kerb.
 - PE identity matmuls accumulate the 49 tap products in PSUM per quarter.
"""
import numpy as np
from contextlib import ExitStack

B, C, H, W = 4, 256, 64, 64
G, K, PAD, R = 16, 7, 3, 4
HW = H * W
P = 128          # partitions / channels per core
NQ = 4           # pixel chunks (quarters; 16 image rows each)
QPIX = HW // NQ  # 1024
QROWS = H // NQ  # 16
NMT = 4          # m-tiles of 16 tap slots (64 slots >= 49 taps)
HP = H + 2 * PAD + 0  # 70 padded rows
WP = W + 2 * PAD      # 70 padded cols

# (q, mt) combos whose kerb goes via the 16-DMA strided replication path;
# the rest use PE selection matmul + ACT copy.  q0 keeps mt0 on the PE path
# so compute starts ~10us earlier (no wait on the first DMA batch).
DMA_MTS = {(q, mt) for q in range(NQ) for mt in (0, 1)}

_CACHE = {}


def _build_nc():
    import concourse.mybir as mybir
    import concourse.tile as tile
    from concourse import bacc

    f32 = mybir.dt.float32
    bf16 = mybir.dt.bfloat16
    nc = bacc.Bacc("TRN2", target_bir_lowering=False, debug=False)

    xpd = nc.dram_tensor("xpd", (P, HP, WP), bf16, kind="ExternalInput")
    xo = nc.dram_tensor("xo", (P, HW), bf16, kind="ExternalInput")
    at = nc.dram_tensor("at", (P, 2, NMT, P), bf16, kind="ExternalInput")
    bias = nc.dram_tensor("bias", (P, NMT), f32, kind="ExternalInput")
    sel = nc.dram_tensor("sel", (P, 16, P), bf16, kind="ExternalInput")
    ident = nc.dram_tensor("ident", (P, P), bf16, kind="ExternalInput")
    out = nc.dram_tensor("out", (P, HW), f32, kind="ExternalOutput")

    with tile.TileContext(nc) as tc:
        with ExitStack() as ctx:
            const = ctx.enter_context(tc.tile_pool(name="const", bufs=1))
            ps_kg = ctx.enter_context(tc.tile_pool(name="ps_kg", bufs=2, space="PSUM"))
            ps_kb = ctx.enter_context(tc.tile_pool(name="ps_kb", bufs=2, space="PSUM"))
            ps_acc = ctx.enter_context(tc.tile_pool(name="ps_acc", bufs=1, space="PSUM"))
            sb_kb = ctx.enter_context(tc.tile_pool(name="sb_kb", bufs=3))
            sb_kb1 = ctx.enter_context(tc.tile_pool(name="sb_kb1", bufs=10))
            sb_prod = ctx.enter_context(tc.tile_pool(name="sb_prod", bufs=12))
            sb_out = ctx.enter_context(tc.tile_pool(name="sb_out", bufs=2))

            xo_sb = const.tile([P, HW], bf16)
            at_sb = const.tile([P, 2, NMT, P], bf16)
            bias_sb = const.tile([P, NMT], f32)
            sel_sb = const.tile([P, 16, P], bf16)
            id_sb = const.tile([P, P], bf16)
            ker_sb = const.tile([P, NMT, HW], bf16)
            xpad = const.tile([P, HP, WP], bf16)

            # small tensors first so ker-gen unblocks asap; xpad arrives
            # pre-padded from the host (no memset / interior copy needed)
            nc.sync.dma_start(at_sb[:], at[:])
            nc.sync.dma_start(bias_sb[:], bias[:])
            nc.sync.dma_start(sel_sb[:], sel[:])
            nc.sync.dma_start(id_sb[:], ident[:])
            # q0 needs only xpad rows 0:24 and xo pixels 0:1024 — load those
            # slices first; the rest is DMA'd after q0's batch DMAs are
            # emitted so it doesn't delay them in the DMA queue
            nc.sync.dma_start(xpad[:, 0:24], xpd[:, 0:24])
            nc.sync.dma_start(xo_sb[:, 0:QPIX], xo[:, 0:QPIX])

            # PE p-state warmup: ~12 throwaway matmuls on zeroed tiles while
            # the input DMAs land, so ker-gen starts at the 2.4 GHz clock.
            wz = const.tile([P, 512], bf16)
            wl = const.tile([P, P], bf16)
            nc.vector.memset(wl[:], 0.0)
            nc.vector.memset(wz[:], 0.0)
            wp = ps_kg.tile([P, 512], f32, name="kg")
            for _ in range(12):
                nc.tensor.matmul(wp[:], wl[:], wz[:], start=True, stop=True)

            # ---- ker-gen: ker_sb[:, mt, :] = (at[:, :, mt].T @ x) + bias ----
            # n-major order so every m-tile's early pixel chunks are ready
            # before the main loop's first quarter starts.
            import concourse.mybir as _mb0

            def emit_kergen(pairs, dve_share):
                for i_kg, (n, mt) in enumerate(pairs):
                    kg = ps_kg.tile([P, 512], f32, name="kg")
                    xq = xpad[:, PAD + 8 * n:PAD + 8 * n + 8, PAD:PAD + W]
                    nc.tensor.matmul(
                        kg[:].rearrange("p (h w) -> p h w", w=W),
                        at_sb[:, 0, mt, :], xq,
                        start=True, stop=False,
                    )
                    nc.tensor.matmul(
                        kg[:], at_sb[:, 1, mt, :],
                        xo_sb[:, n * 512:(n + 1) * 512],
                        start=False, stop=True,
                    )
                    dst = ker_sb[:, mt, n * 512:(n + 1) * 512]
                    if dve_share and i_kg % 2 == 1:
                        # DVE is idle in the prologue: split the critical
                        # first copies so ker-gen drains 2x faster
                        nc.vector.tensor_scalar_add(
                            dst, kg[:], bias_sb[:, mt:mt + 1])
                    else:
                        nc.scalar.add(dst, kg[:], bias_sb[:, mt:mt + 1])

            emit_kergen([(0, 0), (0, 1), (1, 0), (1, 1), (0, 2), (1, 2),
                         (0, 3), (1, 3)], dve_share=True)

            kb_tiles = {}

            def emit_batch(q, mt):
                kb = sb_kb.tile([P, 16, QPIX], bf16, name="kb")
                kb_tiles[(q, mt)] = kb
                src = ker_sb[:, mt, q * QPIX:(q + 1) * QPIX]
                for c16 in range(16):
                    eng = nc.sync if c16 % 2 == 0 else nc.gpsimd
                    eng.dma_start(kb[:][c16::16], src)

            # q0's two batches go out first; the remaining input loads and
            # later ker-gen chunks follow them in the DMA queue
            emit_batch(0, 0)
            emit_batch(0, 1)
            nc.sync.dma_start(xpad[:, 24:HP], xpd[:, 24:HP])
            nc.sync.dma_start(xo_sb[:, QPIX:HW], xo[:, QPIX:HW])
            emit_kergen([(n, mt) for n in range(2, 4) for mt in range(NMT)],
                        dve_share=True)
            emit_kergen([(n, mt) for n in range(4, 8) for mt in range(NMT)],
                        dve_share=False)

            # ---- main loop ----
            import concourse.mybir as _mb

            def xview(q, kk, h0=0, nr=QROWS):
                di, dj = kk // K, kk % K
                r0 = q * QROWS + di + h0
                return xpad[:, r0:r0 + nr, dj:dj + W]

            def emit_quarter(q):
                acc = ps_acc.tile([P, QPIX], f32, name="acc")
                qs = q * QPIX

                # Build the tap schedule for this quarter: list of
                # (kk, src_kind, src) where src_kind is "kb" (DMA-filled
                # batch tile + slot) or "pe" (needs REP matmul + ACT copy).
                # PE-path taps are interleaved among kb-path taps so their
                # serial ACT copies hide under kb-tap TT time.
                kb_blocks, pe_taps = [], []
                for mt in range(NMT):
                    ntap = min(49 - mt * 16, 16)
                    if (q, mt) in DMA_MTS:
                        if (q, mt) not in kb_tiles:
                            emit_batch(q, mt)
                        kb = kb_tiles[(q, mt)]
                        kb_blocks.append(
                            [(mt * 16 + tt, "kb", (kb, tt)) for tt in range(ntap)])
                    else:
                        for tt in range(ntap):
                            pe_taps.append((mt * 16 + tt, "pe", (mt, tt)))
                # pe taps lead each quarter (covering this quarter's batch
                # fill), a few pad the gap between the two kb blocks, and
                # the rest trail; kb blocks stay whole so each batch tile
                # drains early and frees its buffer for a later quarter.
                front, mid = 6, 4
                taps = list(pe_taps[:front])
                if kb_blocks:
                    taps += kb_blocks[0]
                taps += pe_taps[front:front + mid]
                for blk in kb_blocks[1:]:
                    taps += blk
                taps += pe_taps[front + mid:]
                ntot = len(taps)

                # Software-pipelined emission: REP(i+3), copy(i+2), TT(i+1),
                # ACC(i).  Stage state held in dicts keyed by tap index.
                reps = {}   # i -> kps psum tile (PE path only)
                kbs_ = {}   # i -> SBUF bf16 [P, QPIX] kerb for tap i
                prods = {}  # i -> prod tile

                def st_rep(i):
                    kk, kind, s = taps[i]
                    if kind != "pe":
                        return
                    mt, tt = s
                    kps = ps_kb.tile([P, QPIX], f32)
                    for h in range(2):
                        nc.tensor.matmul(
                            kps[:, h * 512:(h + 1) * 512],
                            sel_sb[:, tt, :],
                            ker_sb[:, mt, qs + h * 512:qs + (h + 1) * 512],
                            start=True, stop=True,
                        )
                    reps[i] = kps

                def st_copy(i):
                    kk, kind, s = taps[i]
                    if kind != "pe":
                        return
                    kbs = sb_kb1.tile([P, QPIX], bf16)
                    nc.scalar.copy(kbs[:], reps.pop(i)[:])
                    kbs_[i] = kbs

                def st_tt(i):
                    kk, kind, s = taps[i]
                    if kind == "kb":
                        kb, tt = s
                        in1 = kb[:, tt].rearrange("p (h w) -> p h w", w=W)
                    else:
                        in1 = kbs_.pop(i)[:].rearrange("p (h w) -> p h w", w=W)
                    prod = sb_prod.tile([P, QROWS, W], bf16)
                    nc.vector.tensor_tensor(
                        out=prod[:], in0=xview(q, kk), in1=in1,
                        op=_mb.AluOpType.mult,
                    )
                    prods[i] = prod

                def st_acc(i):
                    pr = prods.pop(i)[:].rearrange("p h w -> p (h w)")
                    for h in range(2):
                        nc.tensor.matmul(
                            acc[:, h * 512:(h + 1) * 512],
                            id_sb[:],
                            pr[:, h * 512:(h + 1) * 512],
                            start=(i == 0), stop=(i == ntot - 1),
                        )

                for i in range(ntot + 3):
                    if i < ntot:
                        st_rep(i)
                    if i - 1 >= 0 and i - 1 < ntot:
                        st_copy(i - 1)
                    if i - 2 >= 0 and i - 2 < ntot:
                        st_tt(i - 2)
                    if i - 3 >= 0:
                        st_acc(i - 3)

                o_sb = sb_out.tile([P, QPIX], f32, name="o_sb")
                nc.scalar.copy(o_sb[:], acc[:])
                # issue via ACT's DGE: an SP-issued DMA here would head-of-
                # line block the next quarter's kb DMAs on the SP sequencer
                nc.scalar.dma_start(out[:, qs:qs + QPIX], o_sb[:])

            for q in range(NQ):
                emit_quarter(q)

    nc.compile()
    return nc


def _host_inputs(x, w_reduce, w_span, b_span):
    import ml_dtypes
    bf = ml_dtypes.bfloat16
    A = (w_span.astype(np.float64) @ w_reduce.astype(np.float64)).astype(np.float32)

    ident = np.eye(P, dtype=bf)
    # sel[r=(g*16+tt), tt, c] = 1 iff r == (c//16)*16 + tt
    sel = np.zeros((P, 16, P), dtype=np.float32)
    for tt in range(16):
        for c in range(P):
            sel[(c // 16) * 16 + tt, tt, c] = 1.0
    sel = sel.astype(bf)

    in_maps = []
    for core in range(8):
        b, half = core // 2, core % 2
        # row layout: m-tile mt, row r = g*16 + tt -> A row (half*8+g)*49 + kk
        # with kk = mt*16 + tt (rows with kk >= 49 are zero-padded)
        Ap = np.zeros((NMT, P, C), dtype=np.float32)
        bp = np.zeros((NMT, P), dtype=np.float32)
        for mt in range(NMT):
            for tt in range(16):
                kk = mt * 16 + tt
                if kk >= K * K:
                    continue
                for g in range(8):
                    r = g * 16 + tt
                    src = (half * 8 + g) * (K * K) + kk
                    Ap[mt, r] = A[src]
                    bp[mt, r] = b_span[src]
        # contraction chunk k holds x channels: chunk 0 = our half, 1 = other
        colperm = np.concatenate([
            np.arange(half * P, (half + 1) * P),
            np.arange((1 - half) * P, (2 - half) * P)])
        Ap = Ap[:, :, colperm]
        # at[cin, k, mt, r] = Ap[mt, r, k*128 + cin]
        at = np.ascontiguousarray(Ap.transpose(2, 0, 1).reshape(2, P, NMT, P)
                                  .transpose(1, 0, 2, 3))
        bias = np.ascontiguousarray(bp.T)  # [P, NMT]

        xh = x[b, half * P:(half + 1) * P]                  # [P, H, W]
        xo_arr = x[b, (1 - half) * P:(2 - half) * P].reshape(P, HW)
        xpd = np.zeros((P, HP, WP), dtype=np.float32)
        xpd[:, PAD:PAD + H, PAD:PAD + W] = xh
        in_maps.append({
            "xpd": xpd.astype(bf),
            "xo": xo_arr.astype(bf),
            "at": at.astype(bf),
            "bias": bias.astype(np.float32),
            "sel": sel,
            "ident": ident,
        })
    return in_maps


def kernel(x, w_reduce, w_span, b_span):
    from concourse import bass_utils
    x = np.asarray(x, dtype=np.float32)
    w_reduce = np.asarray(w_reduce, dtype=np.float32)
    w_span = np.asarray(w_span, dtype=np.float32)
    b_span = np.asarray(b_span, dtype=np.float32)

    if "nc" not in _CACHE:
        _CACHE["nc"] = _build_nc()
    nc = _CACHE["nc"]

    in_maps = _host_inputs(x, w_reduce, w_span, b_span)
    res = bass_utils.run_bass_kernel_spmd(nc, in_maps, core_ids=list(range(8)))

    out = np.empty((B, C, H, W), dtype=np.float32)
    for core in range(8):
        b, half = core // 2, core % 2
        out[b, half * P:(half + 1) * P] = res.results[core]["out"].reshape(P, H, W)
    return out


# revision 39
# speedup vs baseline: 1.0695x; 1.0695x over previous
"""Involution2d Bass kernel for 8 trn2 NeuronCores.

Sharding: core = 2*b + half  (b = batch 0..3, half = group-half 0..1).
Each core computes out[b, half*128:(half+1)*128, :, :].

Math: ker = A @ x[b] + b_span with A = w_span @ w_reduce folded on host.
out[c,p] = sum_kk ker[g(c),kk,p] * xpad[c, p+delta_kk]

Dataflow per core:
 - ker-gen: bf16 PE matmuls (K=256 in 2 chunks) -> PSUM -> ACT copy+bias
   into ker_sb bf16. Row layout per m-tile: r = g*16 + tt (g-major), where
   tap kk = mt*16 + tt.
 - kerb delivery (broadcast each g-row to its 16 channels):
   * DMA path: per (quarter, m-tile), 16 strided SBUF->SBUF DMAs
     (dst[c16::16] <- ker_sb rows) fill kerbS [128, 16, QPIX] bf16.
   * PE path: selection matmul (K=128) -> PSUM -> ACT copy to bf16.
 - DVE tensor_tensor (bf16 2x mode) multiplies shifted xpad view by kerb.
 - PE identity matmuls accumulate the 49 tap products in PSUM per quarter.
"""
import numpy as np
from contextlib import ExitStack

B, C, H, W = 4, 256, 64, 64
G, K, PAD, R = 16, 7, 3, 4
HW = H * W
P = 128          # partitions / channels per core
NQ = 4           # pixel chunks (quarters; 16 image rows each)
QPIX = HW // NQ  # 1024
QROWS = H // NQ  # 16
NMT = 4          # m-tiles of 16 tap slots (64 slots >= 49 taps)
HP = H + 2 * PAD + 0  # 70 padded rows
WP = W + 2 * PAD      # 70 padded cols

# (q, mt) combos whose kerb goes via the 16-DMA strided replication path;
# the rest use PE selection matmul + ACT copy.  q0 keeps mt0 on the PE path
# so compute starts ~10us earlier (no wait on the first DMA batch).
DMA_MTS = {(q, mt) for q in range(NQ) for mt in (0, 1)}

_CACHE = {}


def _build_nc():
    import concourse.mybir as mybir
    import concourse.tile as tile
    from concourse import bacc

    f32 = mybir.dt.float32
    bf16 = mybir.dt.bfloat16
    nc = bacc.Bacc("TRN2", target_bir_lowering=False, debug=False)

    xpd = nc.dram_tensor("xpd", (P, HP, WP), bf16, kind="ExternalInput")
    xo = nc.dram_tensor("xo", (P, HW), bf16, kind="ExternalInput")
    at = nc.dram_tensor("at", (P, 2, NMT, P), bf16, kind="ExternalInput")
    bias = nc.dram_tensor("bias", (P, NMT), f32, kind="ExternalInput")
    sel = nc.dram_tensor("sel", (P, 16, P), bf16, kind="ExternalInput")
    ident = nc.dram_tensor("ident", (P, P), bf16, kind="ExternalInput")
    out = nc.dram_tensor("out", (P, HW), f32, kind="ExternalOutput")

    with tile.TileContext(nc) as tc:
        with ExitStack() as ctx:
            const = ctx.enter_context(tc.tile_pool(name="const", bufs=1))
            ps_kg = ctx.enter_context(tc.tile_pool(name="ps_kg", bufs=2, space="PSUM"))
            ps_kb = ctx.enter_context(tc.tile_pool(name="ps_kb", bufs=2, space="PSUM"))
            ps_acc = ctx.enter_context(tc.tile_pool(name="ps_acc", bufs=1, space="PSUM"))
            sb_kb = ctx.enter_context(tc.tile_pool(name="sb_kb", bufs=3))
            sb_kb1 = ctx.enter_context(tc.tile_pool(name="sb_kb1", bufs=10))
            sb_prod = ctx.enter_context(tc.tile_pool(name="sb_prod", bufs=12))
            sb_out = ctx.enter_context(tc.tile_pool(name="sb_out", bufs=2))

            xo_sb = const.tile([P, HW], bf16)
            at_sb = const.tile([P, 2, NMT, P], bf16)
            bias_sb = const.tile([P, NMT], f32)
            sel_sb = const.tile([P, 16, P], bf16)
            id_sb = const.tile([P, P], bf16)
            ker_sb = const.tile([P, NMT, HW], bf16)
            xpad = const.tile([P, HP, WP], bf16)

            # small tensors first so ker-gen unblocks asap; xpad arrives
            # pre-padded from the host (no memset / interior copy needed)
            nc.sync.dma_start(at_sb[:], at[:])
            nc.sync.dma_start(bias_sb[:], bias[:])
            nc.sync.dma_start(sel_sb[:], sel[:])
            nc.sync.dma_start(id_sb[:], ident[:])
            # q0 needs only xpad rows 0:24 and xo pixels 0:1024 — load those
            # slices first; the rest is DMA'd after q0's batch DMAs are
            # emitted so it doesn't delay them in the DMA queue
            nc.sync.dma_start(xpad[:, 0:24], xpd[:, 0:24])
            nc.sync.dma_start(xo_sb[:, 0:QPIX], xo[:, 0:QPIX])

            # PE p-state warmup: ~12 throwaway matmuls on zeroed tiles while
            # the input DMAs land, so ker-gen starts at the 2.4 GHz clock.
            wz = const.tile([P, 512], bf16)
            wl = const.tile([P, P], bf16)
            nc.vector.memset(wl[:], 0.0)
            nc.vector.memset(wz[:], 0.0)
            wp = ps_kg.tile([P, 512], f32, name="kg")
            for _ in range(12):
                nc.tensor.matmul(wp[:], wl[:], wz[:], start=True, stop=True)

            # ---- ker-gen: ker_sb[:, mt, :] = (at[:, :, mt].T @ x) + bias ----
            # n-major order so every m-tile's early pixel chunks are ready
            # before the main loop's first quarter starts.
            import concourse.mybir as _mb0

            def emit_kergen(pairs, dve_share):
                for i_kg, (n, mt) in enumerate(pairs):
                    kg = ps_kg.tile([P, 512], f32, name="kg")
                    xq = xpad[:, PAD + 8 * n:PAD + 8 * n + 8, PAD:PAD + W]
                    nc.tensor.matmul(
                        kg[:].rearrange("p (h w) -> p h w", w=W),
                        at_sb[:, 0, mt, :], xq,
                        start=True, stop=False,
                    )
                    nc.tensor.matmul(
                        kg[:], at_sb[:, 1, mt, :],
                        xo_sb[:, n * 512:(n + 1) * 512],
                        start=False, stop=True,
                    )
                    dst = ker_sb[:, mt, n * 512:(n + 1) * 512]
                    if dve_share and i_kg % 2 == 1:
                        # DVE is idle in the prologue: split the critical
                        # first copies so ker-gen drains 2x faster
                        nc.vector.tensor_scalar_add(
                            dst, kg[:], bias_sb[:, mt:mt + 1])
                    else:
                        nc.scalar.add(dst, kg[:], bias_sb[:, mt:mt + 1])

            emit_kergen([(0, 0), (0, 1), (1, 0), (1, 1), (0, 2), (1, 2),
                         (0, 3), (1, 3)], dve_share=True)

            kb_tiles = {}

            def emit_batch(q, mt):
                kb = sb_kb.tile([P, 16, QPIX], bf16, name="kb")
                kb_tiles[(q, mt)] = kb
                src = ker_sb[:, mt, q * QPIX:(q + 1) * QPIX]
                for c16 in range(16):
                    eng = nc.sync if c16 % 2 == 0 else nc.gpsimd
                    eng.dma_start(kb[:][c16::16], src)

            # q0's two batches go out first; the remaining input loads and
            # later ker-gen chunks follow them in the DMA queue
            emit_batch(0, 0)
            emit_batch(0, 1)
            nc.sync.dma_start(xpad[:, 24:HP], xpd[:, 24:HP])
            nc.sync.dma_start(xo_sb[:, QPIX:HW], xo[:, QPIX:HW])
            emit_kergen([(n, mt) for n in range(2, 4) for mt in range(NMT)],
                        dve_share=True)
            emit_kergen([(n, mt) for n in range(4, 8) for mt in range(NMT)],
                        dve_share=False)

            # ---- main loop ----
            import concourse.mybir as _mb

            def xview(q, kk, h0=0, nr=QROWS):
                di, dj = kk // K, kk % K
                r0 = q * QROWS + di + h0
                return xpad[:, r0:r0 + nr, dj:dj + W]

            def emit_quarter(q):
                acc = ps_acc.tile([P, QPIX], f32, name="acc")
                qs = q * QPIX

                # Build the tap schedule for this quarter: list of
                # (kk, src_kind, src) where src_kind is "kb" (DMA-filled
                # batch tile + slot) or "pe" (needs REP matmul + ACT copy).
                # PE-path taps are interleaved among kb-path taps so their
                # serial ACT copies hide under kb-tap TT time.
                kb_blocks, pe_taps = [], []
                for mt in range(NMT):
                    ntap = min(49 - mt * 16, 16)
                    if (q, mt) in DMA_MTS:
                        if (q, mt) not in kb_tiles:
                            emit_batch(q, mt)
                        kb = kb_tiles[(q, mt)]
                        kb_blocks.append(
                            [(mt * 16 + tt, "kb", (kb, tt)) for tt in range(ntap)])
                    else:
                        for tt in range(ntap):
                            pe_taps.append((mt * 16 + tt, "pe", (mt, tt)))
                # kb taps in mt-block order; pe taps spread evenly among
                # them (so their serial ACT copies hide under kb-tap TTs)
                kb_taps = [t for blk in kb_blocks for t in blk]
                taps = []
                npe, nkb = len(pe_taps), len(kb_taps)
                rtot = npe + nkb
                ip = ik = 0
                for s in range(rtot):
                    if ip < npe and s * npe >= ip * rtot:
                        taps.append(pe_taps[ip]); ip += 1
                    elif ik < nkb:
                        taps.append(kb_taps[ik]); ik += 1
                    else:
                        taps.append(pe_taps[ip]); ip += 1
                ntot = len(taps)

                # Software-pipelined emission: REP(i+3), copy(i+2), TT(i+1),
                # ACC(i).  Stage state held in dicts keyed by tap index.
                reps = {}   # i -> kps psum tile (PE path only)
                kbs_ = {}   # i -> SBUF bf16 [P, QPIX] kerb for tap i
                prods = {}  # i -> prod tile

                def st_rep(i):
                    kk, kind, s = taps[i]
                    if kind != "pe":
                        return
                    mt, tt = s
                    kps = ps_kb.tile([P, QPIX], f32)
                    for h in range(2):
                        nc.tensor.matmul(
                            kps[:, h * 512:(h + 1) * 512],
                            sel_sb[:, tt, :],
                            ker_sb[:, mt, qs + h * 512:qs + (h + 1) * 512],
                            start=True, stop=True,
                        )
                    reps[i] = kps

                def st_copy(i):
                    kk, kind, s = taps[i]
                    if kind != "pe":
                        return
                    kbs = sb_kb1.tile([P, QPIX], bf16)
                    nc.scalar.copy(kbs[:], reps.pop(i)[:])
                    kbs_[i] = kbs

                def st_tt(i):
                    kk, kind, s = taps[i]
                    if kind == "kb":
                        kb, tt = s
                        in1 = kb[:, tt].rearrange("p (h w) -> p h w", w=W)
                    else:
                        in1 = kbs_.pop(i)[:].rearrange("p (h w) -> p h w", w=W)
                    prod = sb_prod.tile([P, QROWS, W], bf16)
                    nc.vector.tensor_tensor(
                        out=prod[:], in0=xview(q, kk), in1=in1,
                        op=_mb.AluOpType.mult,
                    )
                    prods[i] = prod

                def st_acc(i):
                    pr = prods.pop(i)[:].rearrange("p h w -> p (h w)")
                    for h in range(2):
                        nc.tensor.matmul(
                            acc[:, h * 512:(h + 1) * 512],
                            id_sb[:],
                            pr[:, h * 512:(h + 1) * 512],
                            start=(i == 0), stop=(i == ntot - 1),
                        )

                for i in range(ntot + 3):
                    if i < ntot:
                        st_rep(i)
                    if i - 1 >= 0 and i - 1 < ntot:
                        st_copy(i - 1)
                    if i - 2 >= 0 and i - 2 < ntot:
                        st_tt(i - 2)
                    if i - 3 >= 0:
                        st_acc(i - 3)

                o_sb = sb_out.tile([P, QPIX], f32, name="o_sb")
                nc.scalar.copy(o_sb[:], acc[:])
                # issue via ACT's DGE: an SP-issued DMA here would head-of-
                # line block the next quarter's kb DMAs on the SP sequencer
                nc.scalar.dma_start(out[:, qs:qs + QPIX], o_sb[:])

            for q in range(NQ):
                emit_quarter(q)

    nc.compile()
    return nc


def _host_inputs(x, w_reduce, w_span, b_span):
    import ml_dtypes
    bf = ml_dtypes.bfloat16
    A = (w_span.astype(np.float64) @ w_reduce.astype(np.float64)).astype(np.float32)

    ident = np.eye(P, dtype=bf)
    # sel[r=(g*16+tt), tt, c] = 1 iff r == (c//16)*16 + tt
    sel = np.zeros((P, 16, P), dtype=np.float32)
    for tt in range(16):
        for c in range(P):
            sel[(c // 16) * 16 + tt, tt, c] = 1.0
    sel = sel.astype(bf)

    in_maps = []
    for core in range(8):
        b, half = core // 2, core % 2
        # row layout: m-tile mt, row r = g*16 + tt -> A row (half*8+g)*49 + kk
        # with kk = mt*16 + tt (rows with kk >= 49 are zero-padded)
        Ap = np.zeros((NMT, P, C), dtype=np.float32)
        bp = np.zeros((NMT, P), dtype=np.float32)
        for mt in range(NMT):
            for tt in range(16):
                kk = mt * 16 + tt
                if kk >= K * K:
                    continue
                for g in range(8):
                    r = g * 16 + tt
                    src = (half * 8 + g) * (K * K) + kk
                    Ap[mt, r] = A[src]
                    bp[mt, r] = b_span[src]
        # contraction chunk k holds x channels: chunk 0 = our half, 1 = other
        colperm = np.concatenate([
            np.arange(half * P, (half + 1) * P),
            np.arange((1 - half) * P, (2 - half) * P)])
        Ap = Ap[:, :, colperm]
        # at[cin, k, mt, r] = Ap[mt, r, k*128 + cin]
        at = np.ascontiguousarray(Ap.transpose(2, 0, 1).reshape(2, P, NMT, P)
                                  .transpose(1, 0, 2, 3))
        bias = np.ascontiguousarray(bp.T)  # [P, NMT]

        xh = x[b, half * P:(half + 1) * P]                  # [P, H, W]
        xo_arr = x[b, (1 - half) * P:(2 - half) * P].reshape(P, HW)
        xpd = np.zeros((P, HP, WP), dtype=np.float32)
        xpd[:, PAD:PAD + H, PAD:PAD + W] = xh
        in_maps.append({
            "xpd": xpd.astype(bf),
            "xo": xo_arr.astype(bf),
            "at": at.astype(bf),
            "bias": bias.astype(np.float32),
            "sel": sel,
            "ident": ident,
        })
    return in_maps


def kernel(x, w_reduce, w_span, b_span):
    from concourse import bass_utils
    x = np.asarray(x, dtype=np.float32)
    w_reduce = np.asarray(w_reduce, dtype=np.float32)
    w_span = np.asarray(w_span, dtype=np.float32)
    b_span = np.asarray(b_span, dtype=np.float32)

    if "nc" not in _CACHE:
        _CACHE["nc"] = _build_nc()
    nc = _CACHE["nc"]

    in_maps = _host_inputs(x, w_reduce, w_span, b_span)
    res = bass_utils.run_bass_kernel_spmd(nc, in_maps, core_ids=list(range(8)))

    out = np.empty((B, C, H, W), dtype=np.float32)
    for core in range(8):
        b, half = core // 2, core % 2
        out[b, half * P:(half + 1) * P] = res.results[core]["out"].reshape(P, H, W)
    return out
